# revision 2
# baseline (speedup 1.0000x reference)
"""Trainium2 Bass kernel for nn_patch_expanding.

Computes, for x [32, 1024, 1024] and w [512, 512]:
    xg = x.reshape(B, 32, 32, 1024); x0, x1 = split(xg, channel halves)
    xi = row-interleave(x0, x1) -> [B, 64, 32, 512]
    y  = xi @ w -> reshape [B, 2048, 512]

Data-parallel over batch (4 batches/core on 8 cores); fp16 on device
(hosts rounds inputs; rel err ~5e-4 vs the 2e-2 gate). Per core per rep:
x [4096 tok, 1024 ch] -> y [8192, 512].

- x enters cin-major via HBM->SBUF DMA XBAR transposes [2048,128]->[128,2048]
  (16-bit path; must issue on the SP HWDGE ring -- the ACT ring's xbar
  corrupts data). xt is quad-buffered so the xbar pipeline never stalls.
- Output rows interleave the channel halves at 32-row granularity, so each
  [128, 512] PSUM tile is built from 4 column-tiled concurrent matmuls
  (M=32 quarters, tile_position=(0,32q), quarter q=(i2,s) contracts the
  half-s channels). PSUM partitions then map to 128 contiguous y rows.
- PE: 1024 column-tiled M=32/N=512 fp16 matmuls per rep; 4 quarters stream
  concurrently on separate col groups -> ~the dense 256-matmul roofline.
- DVE evicts PSUM to fp16 (drain-fenced for the store DMA); ACT issues the
  1 MB contiguous stores. Host upcasts y fp16 -> fp32.

fp16 datapath. Per core: x [4096 tok, 1024 ch] -> y [8192, 512].

v6 + engine rebalance: the XBAR transpose's HWDGE descriptor-gen costs
~34ns/16x128-tile (~70us/rep serial), so the 16 transposes per rep are
split across the two HWDGE engines (SP: ch-blocks 0-3, ACT: 4-7); PSUM
evictions move to the otherwise-idle DVE; ACT issues the 1 MB contiguous
stores lagged one load-group so its transposes stay ahead of the PE.
PE: 1024 column-tiled M=32/N=512 fp16 matmuls per rep (4 concurrent
quarters per PSUM tile; partitions = 128 contiguous y rows).
"""
import sys
sys.path.insert(0, "/opt/trn_rl_repo")
import numpy as np

B, L, C = 32, 1024, 1024
NCORES = 8
BPC = B // NCORES
ROWS = BPC * L             # 4096 tokens per core
OROWS = 2 * ROWS
TL = 2048                  # tokens per load-group
NGL = ROWS // TL           # 2 load-groups per rep
WPL = TL // 64             # 32 output windows (128 y rows) per load-group
WPR = ROWS // 64           # 64 windows per rep

_CACHE = {}


def _build(reps: int = 1, sim: bool = False):
    import concourse.bass as bass
    from concourse import mybir

    f16, f32 = mybir.dt.float16, mybir.dt.float32
    nc = bass.Bass(trn_type="TRN2", target_bir_lowering=False, debug=False,
                   num_devices=NCORES)

    xd = nc.dram_tensor("x", [ROWS, C], f16, kind="ExternalInput").ap()
    wd = nc.dram_tensor("w", [512, 512], f16, kind="ExternalInput").ap()
    yd = nc.dram_tensor("y", [OROWS, 512], f16, kind="ExternalOutput").ap()

    s_lw = nc.alloc_semaphore("s_lw")
    s_tr = [nc.alloc_semaphore(f"s_tr{i}") for i in range(4)]
    s_mm = nc.alloc_semaphore("s_mm")    # +1 per completed output window
    s_ye = nc.alloc_semaphore("s_ye")    # +1 per DVE eviction
    s_yd = nc.alloc_semaphore("s_yd")    # +1 per drained store-group
    s_st = [nc.alloc_semaphore("s_st0"), nc.alloc_semaphore("s_st1")]
    all_sems = s_tr + s_st + [s_lw, s_mm, s_ye, s_yd]

    GL = NGL * reps
    W = WPR * reps

    def transposes(eng, lg, kks):
        par, la = lg % 4, lg % NGL
        if lg >= 4:
            # xt[par] free once PE consumed load-group lg-4
            eng.wait_ge(s_mm, WPL * (lg - 3))
        for kk in kks:
            eng.dma_start(
                xt_a[:, par, kk, :],
                xd[TL * la:TL * la + TL, 128 * kk:128 * kk + 128],
                transpose=True,
            ).then_inc(s_tr[par], 16)

    with (
        nc.sbuf_tensor("xt", [128, 4, 8, TL], f16) as xt,
        nc.sbuf_tensor("wsb", [128, 4, 512], f16) as wsb,
        nc.sbuf_tensor("ysb", [128, 2, 8, 512], f16) as ysb,
        nc.psum_tensor("ps", [128, 8, 512], f32) as ps,
    ):
        xt_a, wsb_a, ysb_a, ps_a = xt.ap(), wsb.ap(), ysb.ap(), ps.ap()

        if not sim:
            for s in all_sems:
                nc.gpsimd.sem_clear(s)
            for eng in (nc.sync, nc.tensor, nc.vector, nc.scalar):
                for _ in range(4):
                    eng.nop(cycle_cnt=6000, nofuse=True)

        with nc.Block() as block:

            @block.gpsimd
            def _(g):
                g.wait_ge(s_st[0], 16 * (W // 16))
                g.wait_ge(s_st[1], 16 * (W // 16))
                if not sim:
                    for s in all_sems:
                        g.sem_clear(s)

            @block.sync
            def _(sp):
                sp.dma_start(wsb_a[:], wd.rearrange("(kk p) n -> p kk n", p=128)
                             ).then_inc(s_lw, 16)
                for lg in range(GL):
                    transposes(sp, lg, range(8))

            @block.scalar
            def _(ac):
                for lg in range(GL + 1):
                    if lg >= 1:
                        for sg in range(4 * (lg - 1), 4 * lg):
                            parS = sg % 2
                            ac.wait_ge(s_ye, 8 * (sg + 1))
                            ac.wait_ge(s_yd, sg + 1)
                            ybase = 1024 * (sg % (WPR // 8))
                            ac.dma_start(
                                yd[ybase:ybase + 1024, :].rearrange(
                                    "(mm p) n -> p mm n", p=128),
                                ysb_a[:, parS, :, :],
                            ).then_inc(s_st[parS], 16)

            @block.tensor
            def _(pe):
                pe.wait_ge(s_lw, 16)
                for widx in range(W):
                    lg, m = widx // WPL, widx % WPL
                    par, bank = lg % 4, widx % 8
                    if m == 0:
                        pe.wait_ge(s_tr[par], 128 * (lg // 4 + 1))
                    if widx >= 8:
                        pe.wait_ge(s_ye, widx - 7)   # psum bank free
                    for k in range(4):
                        for q in range(4):
                            i2, s = q // 2, q % 2
                            tb = 64 * m + 32 * i2
                            inst = pe.matmul(
                                ps_a[32 * q:32 * q + 32, bank, :],
                                xt_a[:, par, 4 * s + k, tb:tb + 32],
                                wsb_a[:, k, :],
                                start=(k == 0), stop=(k == 3),
                                tile_position=(0, 32 * q),
                            )
                            if k == 3 and q == 3:
                                inst.then_inc(s_mm)

            @block.vector
            def _(dv):
                for widx in range(W):
                    slot, sg = widx % 8, widx // 8
                    parS = sg % 2
                    if slot == 0 and sg >= 2:
                        dv.wait_ge(s_st[parS], 16 * (sg // 2))  # ysb[parS] free
                    dv.wait_ge(s_mm, widx + 1)
                    dv.tensor_copy(ysb_a[:, parS, slot, :], ps_a[:, slot, :]
                                   ).then_inc(s_ye)
                    if slot == 7:
                        # visibility barrier for the ACT store of this group
                        dv.drain().then_inc(s_yd)

    return nc


def _in_maps(x: np.ndarray, w: np.ndarray) -> list:
    xs = np.ascontiguousarray(x, dtype=np.float16).reshape(NCORES, ROWS, C)
    wh = np.ascontiguousarray(w, dtype=np.float16)
    return [{"x": xs[i], "w": wh} for i in range(NCORES)]


def kernel(x: np.ndarray, w: np.ndarray) -> np.ndarray:
    from concourse.bass_utils import run_bass_kernel_spmd

    if "nc" not in _CACHE:
        _CACHE["nc"] = _build()
    nc = _CACHE["nc"]

    in_maps = _in_maps(x, w)
    res = run_bass_kernel_spmd(nc, in_maps, list(range(NCORES)))
    y = np.stack([res.results[i]["y"] for i in range(NCORES)], axis=0)
    return y.reshape(B, 2 * L, C // 2).astype(np.float32)
